# revision 1
# baseline (speedup 1.0000x reference)
"""DTranNER CRF loss kernel for Trainium2 (8 NeuronCores, data-parallel over batch).

Strategy
--------
Batch (B=256) is sharded 8 ways (32 sentences/core).  Each core computes, for
its sentences:

* pairwise CRF log-partition alpha_pp: a 511-step log-semiring scan over the
  streamed feats_pp [b,t,24,24] matrices, run in *factored linear space*
  (state u = exp(fv - s); scalar log-scale s accumulated lazily every R
  steps).  The scan is split into a forward chain (t=0..255) and a backward
  chain (t=510..256) that meet in the middle -- two independent chains halve
  the sequential-latency wall.
* unary CRF log-partition alpha: same recurrence with the constant
  transitions matrix as stationary weights on the tensor engine; forward and
  backward unary chains are packed into one [64, b] tile so each slot is a
  single matmul + a single vector multiply.
* gold-path scores: operand values host-gathered (pure data movement, like
  the other layout transforms); all arithmetic (sums) happens on device.

Per pairwise step (per chain): DVE bf16 multiply (state broadcast along the
outer free dim via a step-0 AP) + segmented X-reduce; the tensor engine
replicates the fragmented reduce output across the 4 partition groups with
constant 0/1 selector matmuls; ACT copies PSUM->SBUF (folding in the
occasional 1/z renorm scale).  The fp32 HBM stream is exp'ed in bulk on ACT
into bf16 tiles.  Host-side prep is layout-only (slicing / transposing).
"""

import numpy as np
import ml_dtypes
from contextlib import ExitStack

import concourse.bass as bass
import concourse.bacc as bacc
import concourse.tile as tile
from concourse import mybir
from concourse.bass_utils import run_bass_kernel_spmd

FP = mybir.dt.float32
BF = mybir.dt.bfloat16
I32 = mybir.dt.int32

B, T, K = 256, 512, 24
START, STOP = 22, 23
NCORES = 8
N1, N2 = 4, 6  # K = N1*N2 partition/free split

AF = mybir.ActivationFunctionType
ALU = mybir.AluOpType
AX = mybir.AxisListType


class _P:
    """Container for build-time params + pools."""



def _pairwise_step(nc, p, E_slice3, state, scale_ap):
    PP = p.PP
    prod = p.sb.tile([PP, N2, K], BF, tag="prod")
    u_b = state["u"][:, :].unsqueeze(1).broadcast_to([PP, N2, K])
    nc.vector.tensor_tensor(out=prod[:], in0=E_slice3, in1=u_b, op=ALU.mult)
    frag = p.sb.tile([PP, N2], BF, tag="frag")
    with nc.allow_low_precision("bf16 CRF inner state"):
        nc.vector.tensor_reduce(out=frag[:], in_=prod[:], axis=AX.X, op=ALU.add)
    urep_ps = p.ps2.tile([PP, K], FP, tag="urep")
    for k in range(N1):
        nc.tensor.matmul(
            out=urep_ps[:, k * N2 : (k + 1) * N2],
            lhsT=p.selw_sb[:, k * PP : (k + 1) * PP],
            rhs=frag[:], start=True, stop=True,
        )
    if scale_ap is not None:
        nc.vector.tensor_scalar(out=urep_ps[:], in0=urep_ps[:], scalar1=scale_ap, scalar2=None, op0=ALU.mult)
    state["u"] = urep_ps
    state["urep_ps"] = urep_ps


def _pairwise_renorm(nc, p, state, zbuf, slot):
    """z = sum(urep) -> zbuf[:, slot]; return 1/z (folded into next copy).
    All the ln() calls happen in one batched pass at the end."""
    PP = p.PP
    nc.vector.tensor_reduce(
        out=zbuf[:, slot : slot + 1], in_=state["urep_ps"][:], axis=AX.X, op=ALU.add
    )
    rz = p.sb.tile([PP, 1], FP, tag="rz")
    nc.vector.reciprocal(out=rz[:], in_=zbuf[:, slot : slot + 1])
    return rz[:]


def build_kernel(BC=32, TT=512, TC=32, R=8):
    """Build the per-core Bass program.  BC = sentences per core."""
    PP = BC * N1
    H = TT // 2            # forward pairwise steps (matrices t = 0..H-1)
    HB = TT - 1 - H        # backward pairwise steps (matrices t = TT-2..H)
    UROW = 64              # unary packing: rows 0..K fwd, 32..32+K bwd
    SL = H                 # slots
    NF = N2 * K            # 144
    CP = 3.8               # pairwise exp pre-scale (exp(x-CP))
    CU = 3.8               # unary exp pre-scale
    RW = UROW * BC         # ftp2 row stride

    nc = bacc.Bacc("TRN2", target_bir_lowering=False)
    fppF = nc.dram_tensor("fppF", [BC, N1, H, NF], FP, kind="ExternalInput")
    fppB = nc.dram_tensor("fppB", [BC, N1, HB, NF], FP, kind="ExternalInput")
    winit = nc.dram_tensor("winit", [BC, K], FP, kind="ExternalInput")
    ftp2 = nc.dram_tensor("ftp2", [SL, UROW, BC], FP, kind="ExternalInput")
    eflast = nc.dram_tensor("eflast", [K, BC], FP, kind="ExternalInput")
    transT = nc.dram_tensor("transT", [K, K], FP, kind="ExternalInput")
    transO = nc.dram_tensor("transO", [K, K], FP, kind="ExternalInput")
    gvals = nc.dram_tensor("gvals", [BC, 3 * TT + 4], FP, kind="ExternalInput")
    selw = nc.dram_tensor("selw", [PP, N1 * PP], BF, kind="ExternalInput")
    nll = nc.dram_tensor("nll", [BC], FP, kind="ExternalOutput")
    scr = nc.dram_tensor("scratch", [4, BC], FP)

    p = _P()
    p.PP = PP

    with tile.TileContext(nc) as tc, ExitStack() as ctx:
        p.sb = ctx.enter_context(tc.tile_pool(name="sb", bufs=3))
        p.ps2 = ctx.enter_context(tc.tile_pool(name="ps2", bufs=2, space="PSUM"))
        p.ps1 = ctx.enter_context(tc.tile_pool(name="ps1", bufs=1, space="PSUM"))
        big = ctx.enter_context(tc.tile_pool(name="big", bufs=2))
        ebig = ctx.enter_context(tc.tile_pool(name="ebig", bufs=2))
        per = ctx.enter_context(tc.tile_pool(name="per", bufs=1))
        sb, ps1, ps2 = p.sb, p.ps1, p.ps2

        # ---------------- constants ----------------
        cpb = per.tile([128, 1], FP, tag="cpb")
        nc.vector.memset(cpb[:], -CP)
        cub = per.tile([128, 1], FP, tag="cub")
        nc.vector.memset(cub[:], -CU)
        selw_sb = per.tile([PP, N1 * PP], BF, tag="selw")
        nc.sync.dma_start(out=selw_sb[:], in_=selw[:])
        p.selw_sb = selw_sb

        # Unary stationary weights, block matrix [UROW, UROW]:
        #   rows 0..K,  cols 0..K  : exp(transT)[p, n]   (fwd)
        #   rows 32.., cols 32..   : exp(transO)[n, p]   (bwd)
        uwst1 = per.tile([K, K], FP, tag="uwst1")
        nc.sync.dma_start(out=uwst1[:], in_=transT[:])
        uwst2 = per.tile([UROW, K], FP, tag="uwst2")
        nc.sync.dma_start(out=uwst2[32 : 32 + K, :], in_=transO[:])
        uw = per.tile([UROW, UROW], BF, tag="uw")
        nc.vector.memset(uw[:], 0.0)
        nc.scalar.activation(out=uw[0:K, 0:K], in_=uwst1[:], func=AF.Exp)
        nc.scalar.activation(
            out=uw[32 : 32 + K, 32 : 32 + K], in_=uwst2[32 : 32 + K, :], func=AF.Exp
        )

        uones = per.tile([UROW, 2], BF, tag="uones")
        nc.vector.memset(uones[:], 0.0)
        nc.vector.memset(uones[0:K, 0:1], 1.0)
        nc.vector.memset(uones[32 : 32 + K, 1:2], 1.0)
        # usel [2, UROW]: row 0 selects fwd rows, row 1 selects bwd rows.
        # Row 1 can't be written by compute (start partition 1) -> DMA bounce.
        usel = per.tile([2, UROW], BF, tag="usel")
        nc.vector.memset(usel[:], 0.0)
        nc.vector.memset(usel[0:1, 0:K], 1.0)
        rowB = sb.tile([1, UROW], BF, tag="rowB")
        nc.vector.memset(rowB[:], 0.0)
        nc.vector.memset(rowB[0:1, 32 : 32 + K], 1.0)
        nc.sync.dma_start(out=usel[1:2, :], in_=rowB[:])
        # ones [2,1] to sum the two scale rows at the end
        ones2 = per.tile([2, 1], FP, tag="ones2")
        nc.vector.memset(ones2[:], 1.0)

        tc.strict_bb_all_engine_barrier()

        # ---------------- unary Ef table ----------------
        # eft layout: [UROW, SL*BC], slot g at free offset g*BC
        eft = per.tile([UROW, SL * BC], BF, tag="eft")
        nchunk = 4 if SL >= 4 else 1
        cs2 = SL // nchunk
        cstep = cs2 * BC
        src = ftp2[:, :, :].rearrange("s r j -> r s j")
        for c in range(nchunk):
            ftile = big.tile([UROW, cstep], FP, tag="ftp_in")
            nc.sync.dma_start(
                out=ftile[:].rearrange("p (s j) -> p s j", j=BC),
                in_=src[:, c * cs2 : (c + 1) * cs2, :],
            )
            nc.scalar.activation(
                out=eft[:, c * cstep : (c + 1) * cstep], in_=ftile[:], func=AF.Exp, bias=cub[0:UROW, :]
            )

        # ---------------- state init ----------------
        uf0 = per.tile([PP, K], BF, tag="uf0")
        nc.vector.memset(uf0[:], 0.0)
        nc.vector.memset(uf0[:, START : START + 1], 1.0)
        NRN = (H + R - 1) // R + 1
        zbufF = per.tile([PP, NRN], FP, tag="zbufF")
        nc.vector.memset(zbufF[:], 1.0)
        zbufB = per.tile([PP, NRN], FP, tag="zbufB")
        nc.vector.memset(zbufB[:], 1.0)
        zbufU = per.tile([2, NRN * BC], FP, tag="zbufU")
        nc.vector.memset(zbufU[:], 1.0)
        stF = {"u": uf0}

        wfrag_f = sb.tile([PP, N2], FP, tag="wfrag_f")
        nc.sync.dma_start(
            out=wfrag_f[:], in_=winit[:, :].rearrange("b (p1 p2) -> (b p1) p2", p1=N1)
        )
        wfrag = sb.tile([PP, N2], BF, tag="wfrag")
        nc.scalar.activation(out=wfrag[:], in_=wfrag_f[:], func=AF.Exp, bias=cpb[0:PP, :])
        ub_ps = ps1.tile([PP, K], FP, tag="pmisc")
        for k in range(N1):
            nc.tensor.matmul(
                out=ub_ps[:, k * N2 : (k + 1) * N2],
                lhsT=selw_sb[:, k * PP : (k + 1) * PP],
                rhs=wfrag[:],
                start=True,
                stop=True,
            )
        ub0 = per.tile([PP, K], BF, tag="ub0")
        nc.scalar.activation(out=ub0[:], in_=ub_ps[:], func=AF.Copy)
        stB = {"u": ub0}

        # unary state [UROW, BC]
        us0 = per.tile([UROW, BC], BF, tag="us0")
        nc.vector.memset(us0[:], 0.0)
        row1 = sb.tile([1, BC], BF, tag="row1")
        nc.vector.memset(row1[:], 1.0)
        nc.sync.dma_start(out=us0[START : START + 1, :], in_=row1[:])
        tstop = sb.tile([UROW, 1], FP, tag="tstop")
        nc.sync.dma_start(
            out=tstop[32 : 32 + K, :],
            in_=transO[STOP : STOP + 1, :].rearrange("o k -> k o"),
        )
        tstop_e = sb.tile([UROW, 1], BF, tag="tstop_e")
        nc.scalar.activation(out=tstop_e[32 : 32 + K, :], in_=tstop[32 : 32 + K, :], func=AF.Exp)
        nc.vector.tensor_copy(
            out=us0[32 : 32 + K, :], in_=tstop_e[32 : 32 + K, :].broadcast_to([K, BC])
        )
        stU = us0

        tc.strict_bb_all_engine_barrier()

        # ---------------- gold-path score values (host-gathered operands) ----
        gv = per.tile([BC, 3 * TT + 4], FP, tag="gv")
        nc.sync.dma_start(out=gv[:], in_=gvals[:])

        # ---------------- main streamed loop ----------------
        ntiles = (H + TC - 1) // TC
        rzF = rzB = None
        nF = nB = nU = 0
        for it in range(ntiles):
            t0 = it * TC
            ntF = min(TC, H - t0)
            ntB = min(TC, HB - t0)
            ftile = big.tile([PP, TC * NF], FP, tag="ftileF")
            nc.sync.dma_start(
                out=ftile[:, 0 : ntF * NF],
                in_=fppF[:, :, t0 : t0 + ntF, :].rearrange("b n t f -> (b n) (t f)"),
            )
            eF = ebig.tile([PP, TC * NF], BF, tag="eF")
            nc.scalar.activation(out=eF[:, 0 : ntF * NF], in_=ftile[:, 0 : ntF * NF], func=AF.Exp, bias=cpb[0:PP, :])
            if ntB > 0:
                btile = big.tile([PP, TC * NF], FP, tag="ftileB")
                nc.sync.dma_start(
                    out=btile[:, 0 : ntB * NF],
                    in_=fppB[:, :, t0 : t0 + ntB, :].rearrange("b n t f -> (b n) (t f)"),
                )
                eB = ebig.tile([PP, TC * NF], BF, tag="eB")
                nc.scalar.activation(out=eB[:, 0 : ntB * NF], in_=btile[:, 0 : ntB * NF], func=AF.Exp, bias=cpb[0:PP, :])

            for m in range(ntF):
                eF3 = eF[:, m * NF : (m + 1) * NF].rearrange("q (a b) -> q a b", a=N2)
                _pairwise_step(nc, p, eF3, stF, rzF)
                rzF = None
                nF += 1
                if m < ntB:
                    eB3 = eB[:, m * NF : (m + 1) * NF].rearrange("q (a b) -> q a b", a=N2)
                    _pairwise_step(nc, p, eB3, stB, rzB)
                    rzB = None
                    nB += 1

                # ---- unary slot: mul-first then matvec ----
                g = nU
                ef_sl = eft[:, g * BC : (g + 1) * BC]
                us_m = sb.tile([UROW, BC], BF, tag="us_m")
                nc.vector.tensor_tensor(out=us_m[:], in0=stU[:], in1=ef_sl, op=ALU.mult)
                vu_ps = ps2.tile([UROW, BC], FP, tag="vu")
                nc.tensor.matmul(out=vu_ps[:], lhsT=uw[:], rhs=us_m[:], start=True, stop=True)
                stU = vu_ps
                nU += 1

                # ---- lazy renorms ----
                if nF % R == 0 and nF < H:
                    rzF = _pairwise_renorm(nc, p, stF, zbufF, nF // R)
                if nB > 0 and nB % R == 0 and nB < HB and m < ntB:
                    rzB = _pairwise_renorm(nc, p, stB, zbufB, nB // R)
                if nU % R == 0 and nU < SL:
                    us_c = sb.tile([UROW, BC], BF, tag="us_c")
                    nc.scalar.activation(out=us_c[:], in_=stU[:], func=AF.Copy)
                    stU = us_c
                    zu_ps = ps1.tile([2, BC], FP, tag="pmisc")
                    nc.tensor.matmul(out=zu_ps[:], lhsT=uones[:], rhs=stU[:], start=True, stop=True)
                    zsl = zbufU[:, (nU // R) * BC : (nU // R + 1) * BC]
                    nc.vector.tensor_copy(out=zsl, in_=zu_ps[:])
                    rzu = sb.tile([2, BC], FP, tag="rzu")
                    nc.vector.reciprocal(out=rzu[:], in_=zu_ps[:])
                    rzu_b = sb.tile([2, BC], BF, tag="rzu_b")
                    nc.vector.tensor_copy(out=rzu_b[:], in_=rzu[:])
                    rzu_rep = ps1.tile([UROW, BC], FP, tag="pmisc")
                    nc.tensor.matmul(out=rzu_rep[:], lhsT=usel[:], rhs=rzu_b[:], start=True, stop=True)
                    rzu_s = sb.tile([UROW, BC], BF, tag="rzu_s")
                    nc.scalar.activation(out=rzu_s[:], in_=rzu_rep[:], func=AF.Copy)
                    us_sc = sb.tile([UROW, BC], BF, tag="us_s")
                    nc.vector.tensor_tensor(out=us_sc[:], in0=stU[:], in1=rzu_s[:], op=ALU.mult)
                    stU = us_sc

        # ---------------- tails ----------------
        # batched ln of all buffered renorm z values, then sum per chain
        lzF = sb.tile([PP, NRN], FP, tag="lzF")
        nc.scalar.activation(out=lzF[:], in_=zbufF[:], func=AF.Ln)
        sF = sb.tile([PP, 1], FP, tag="sF")
        nc.vector.tensor_reduce(out=sF[:], in_=lzF[:], axis=AX.X, op=ALU.add)
        lzB = sb.tile([PP, NRN], FP, tag="lzB")
        nc.scalar.activation(out=lzB[:], in_=zbufB[:], func=AF.Ln)
        sB = sb.tile([PP, 1], FP, tag="sB")
        nc.vector.tensor_reduce(out=sB[:], in_=lzB[:], axis=AX.X, op=ALU.add)
        lzU = sb.tile([2, NRN * BC], FP, tag="lzU")
        nc.scalar.activation(out=lzU[:], in_=zbufU[:], func=AF.Ln)
        sU = sb.tile([2, BC], FP, tag="sU")
        nc.vector.tensor_reduce(
            out=sU[:],
            in_=lzU[:].rearrange("a (s b) -> a b s", b=BC),
            axis=AX.X,
            op=ALU.add,
        )
        # pairwise meet
        ufc = sb.tile([PP, K], BF, tag="ufc")
        nc.scalar.activation(out=ufc[:], in_=stF["u"][:], func=AF.Copy)
        pm = sb.tile([PP, K], FP, tag="pmeet")
        nc.vector.tensor_tensor(out=pm[:], in0=ufc[:], in1=stB["u"][:], op=ALU.mult)
        qq = sb.tile([PP, 1], FP, tag="qq")
        nc.vector.tensor_reduce(out=qq[:], in_=pm[:], axis=AX.X, op=ALU.add)
        lq = sb.tile([PP, 1], FP, tag="lq")
        nc.scalar.activation(out=lq[:], in_=qq[:], func=AF.Ln)
        nc.vector.tensor_tensor(out=lq[:], in0=lq[:], in1=sF[:], op=ALU.add)
        nc.vector.tensor_tensor(out=lq[:], in0=lq[:], in1=sB[:], op=ALU.add)
        nc.vector.tensor_scalar(out=lq[:], in0=lq[:], scalar1=CP * (H + HB + 1), scalar2=None, op0=ALU.add)
        nc.sync.dma_start(
            out=scr[0:1, :],
            in_=lq[:, :].rearrange("(b n) o -> b (n o)", n=N1)[:, 0:1],
        )

        # unary meet (incl. deferred Ef_{H-1} diag factor)
        efl = sb.tile([K, BC], FP, tag="efl")
        nc.sync.dma_start(out=efl[:], in_=eflast[:])
        efl_e = sb.tile([K, BC], BF, tag="efl_e")
        nc.scalar.activation(out=efl_e[:], in_=efl[:], func=AF.Exp)
        ustail = sb.tile([UROW, BC], BF, tag="ustail")
        nc.scalar.activation(out=ustail[:], in_=stU[:], func=AF.Copy)
        stU = ustail
        usb_c = sb.tile([K, BC], BF, tag="usb_c")
        nc.sync.dma_start(out=usb_c[:], in_=stU[32 : 32 + K, :])
        um = sb.tile([K, BC], BF, tag="umeet")
        nc.vector.tensor_tensor(out=um[:], in0=stU[0:K, :], in1=usb_c[:], op=ALU.mult)
        nc.vector.tensor_tensor(out=um[:], in0=um[:], in1=efl_e[:], op=ALU.mult)
        ones_k = sb.tile([K, 1], BF, tag="ones_k")
        nc.vector.memset(ones_k[:], 1.0)
        au_ps = ps1.tile([1, BC], FP, tag="pmisc")
        nc.tensor.matmul(out=au_ps[:], lhsT=ones_k[:], rhs=um[:], start=True, stop=True)
        lau = sb.tile([1, BC], FP, tag="lau")
        nc.scalar.activation(out=lau[:], in_=au_ps[:], func=AF.Ln)
        su_ps = ps1.tile([1, BC], FP, tag="pmisc")
        nc.tensor.matmul(out=su_ps[:], lhsT=ones2[:], rhs=sU[:], start=True, stop=True)
        nc.vector.tensor_tensor(out=lau[:], in0=lau[:], in1=su_ps[:], op=ALU.add)
        nc.vector.tensor_scalar(out=lau[:], in0=lau[:], scalar1=CU * (2 * SL), scalar2=None, op0=ALU.add)
        nc.sync.dma_start(out=scr[1:2, :], in_=lau[:])

        # score reduction (single fused sum of all gold-path terms)
        sc = sb.tile([BC, 1], FP, tag="sc")
        nc.vector.tensor_reduce(out=sc[:], in_=gv[:], axis=AX.X, op=ALU.add)

        app = sb.tile([BC, 1], FP, tag="app")
        nc.sync.dma_start(out=app[:], in_=scr[0:1, :].rearrange("o b -> b o"))
        alu_ = sb.tile([BC, 1], FP, tag="alu")
        nc.sync.dma_start(out=alu_[:], in_=scr[1:2, :].rearrange("o b -> b o"))

        res = sb.tile([BC, 1], FP, tag="res")
        nc.vector.tensor_tensor(out=res[:], in0=app[:], in1=alu_[:], op=ALU.add)
        nc.vector.tensor_tensor(out=res[:], in0=res[:], in1=sc[:], op=ALU.subtract)
        nc.sync.dma_start(out=nll[:], in_=res[:].rearrange("b o -> (b o)"))

    nc.compile()
    return nc


# ======================= host-side prep =======================

def prep_core_inputs(feats, fpp, transitions, tags, b0, BC, TT):
    """Build the per-core input map (pure layout transforms)."""
    H = TT // 2
    HB = TT - 1 - H
    fe = feats[b0 : b0 + BC]          # [BC, T, K]
    fp = fpp[b0 : b0 + BC]            # [BC, T, K*K]
    tg = tags[b0 : b0 + BC]           # [BC, T]
    fp4 = fp.reshape(BC, TT, K, K)    # [b, t, n, p]

    fwd = fp4[:, 0:H].reshape(BC, H, N1, N2, K).transpose(0, 2, 1, 3, 4)
    fppF = np.ascontiguousarray(fwd.reshape(BC, N1, H, N2 * K), np.float32)

    # bwd slot s holds matrix t = TT-2-s, (p-major) transposed
    bwd_t = fp4[:, H : TT - 1][:, ::-1]            # [b, s, n, p]
    bwd = bwd_t.transpose(0, 1, 3, 2)              # [b, s, p, n]
    bwd = bwd.reshape(BC, HB, N1, N2, K).transpose(0, 2, 1, 3, 4)
    fppB = np.ascontiguousarray(bwd.reshape(BC, N1, HB, N2 * K), np.float32)

    winit = np.ascontiguousarray(fp4[:, TT - 1, STOP, :], np.float32)

    # unary Ef table: fwd rows at slot s hold feats[t=s-1] (slot 0 = zeros);
    # bwd rows at slot s hold feats[t=TT-1-s]
    ftp2 = np.zeros((H, 64, BC), np.float32)
    ftp2[1:, 0:K, :] = fe[:, 0 : H - 1].transpose(1, 2, 0)
    ftp2[:, 32 : 32 + K, :] = fe[:, TT - 1 : H - 1 : -1].transpose(1, 2, 0)
    eflast = np.ascontiguousarray(fe[:, H - 1, :].T, np.float32)  # [K, BC]

    # gold-path score operands (gather = data movement; summation on device)
    tgi = np.asarray(tg, np.int64)
    te = np.concatenate([np.full((BC, 1), START, np.int64), tgi,
                         np.full((BC, 1), STOP, np.int64)], axis=1)  # [BC, TT+2]
    nxt, prv = te[:, 1:], te[:, :-1]                                  # [BC, TT+1]
    b_ = np.arange(BC)[:, None]
    t_ = np.arange(TT)[None, :]
    gvals = np.zeros((BC, 3 * TT + 4), np.float32)
    gvals[:, 0 : TT + 1] = transitions[nxt, prv]
    gvals[:, TT + 1 : 2 * TT + 1] = np.take_along_axis(
        fe, tgi[:, :, None], axis=2)[..., 0]
    gvals[:, 2 * TT + 1 : 3 * TT + 1] = fp4[b_, np.minimum(t_, TT - 2),
                                            nxt[:, 0:TT], prv[:, 0:TT]]
    # overwrite the t = TT-1 pp term with the terminal fpp[., TT-1, STOP, tags[-1]]
    gvals[:, 3 * TT] = fp4[np.arange(BC), TT - 1, STOP, tgi[:, -1]]
    gvals[:, 3 * TT - 1] = fp4[np.arange(BC), TT - 2, nxt[:, TT - 2], prv[:, TT - 2]]

    PP = BC * N1
    selw = np.zeros((PP, N1, PP), np.float32)
    b_idx = np.arange(BC)
    for k in range(N1):
        for n1p in range(N1):
            selw[b_idx * N1 + k, k, b_idx * N1 + n1p] = 1.0
    selw = selw.reshape(PP, N1 * PP).astype(ml_dtypes.bfloat16)

    return {
        "fppF": fppF,
        "fppB": fppB,
        "winit": winit,
        "ftp2": ftp2,
        "eflast": eflast,
        "transT": np.ascontiguousarray(transitions.T, np.float32),
        "transO": np.ascontiguousarray(transitions, np.float32),
        "gvals": gvals,
        "selw": selw,
    }


_NC_CACHE = {}


def get_nc(BC, TT, TC=32, R=8):
    key = (BC, TT, TC, R)
    if key not in _NC_CACHE:
        _NC_CACHE[key] = build_kernel(BC=BC, TT=TT, TC=TC, R=R)
    return _NC_CACHE[key]


def kernel(feats, feats_pp, transitions, tags):
    feats = np.asarray(feats, np.float32)
    feats_pp = np.asarray(feats_pp, np.float32)
    transitions = np.asarray(transitions, np.float32)
    tags_np = np.asarray(tags)

    BC = B // NCORES
    nc = get_nc(BC, T)
    in_maps = [
        prep_core_inputs(feats, feats_pp, transitions, tags_np, c * BC, BC, T)
        for c in range(NCORES)
    ]
    r = run_bass_kernel_spmd(nc, in_maps, list(range(NCORES)))
    out = np.concatenate([r.results[c]["nll"] for c in range(NCORES)])
    return out.astype(np.float32)


if __name__ == "__main__":
    rng = np.random.default_rng(0)
    feats = rng.standard_normal((B, T, K), dtype=np.float32)
    fpp = rng.standard_normal((B, T, K * K), dtype=np.float32)
    tr = rng.standard_normal((K, K), dtype=np.float32)
    tr[START, :] = -100.0
    tr[:, STOP] = -100.0
    tags = rng.integers(0, K - 2, size=(B, T)).astype(np.int32)
    out = kernel(feats, fpp, tr, tags)
    print(out.shape, out[:4])

